# revision 1
# baseline (speedup 1.0000x reference)
"""DGCNN forward pass on Trainium2 — Bass/Tile kernel, 8-core data parallel.

Sharding: 16 graphs over 8 cores (2 graphs/core). All compute per graph is
local to one core; host concatenates the per-core [2, 1024] outputs.

Per-layer math (per graph), equivalent to the reference EdgeConv:
    a   = x @ (W[:C] - W[C:]) + b            # [n, O]
    c   = x @ W[C:]                          # [n, O]
    t   = 2 * x @ x^T - |x_j|^2              # kNN ranking score (max-top-k)
    idx = top-20 of t per row
    m_i = max_k c[idx[i, k]]
    x'  = leaky_relu(a + m, 0.02)
since max_k leaky(a_i + c_jk) == leaky(a_i + max_k c_jk) (monotone) and the
|x_i|^2 term of the squared distance is constant per row (rank-invariant).

All tensors are kept in channel-major layout ([channels, points]). The
-|x_j|^2 term is folded into the t-matmul as an extra accumulate pass with an
all-(-1) stationary operand against sq = x*x.

Top-20 selection per 128-row tile: 16x max8 over 128-wide segments, merge
candidates with max8/match_replace rounds, then max_index against the full
row for global indices. Neighbor max-aggregation gathers c rows with GPSIMD
ap_gather (SBUF-to-SBUF) per k, then running elementwise max.
"""

import sys
from contextlib import ExitStack

import numpy as np

sys.path.insert(0, "/opt/trn_rl_repo")

import concourse.bass as bass
from concourse import bacc
import concourse.mybir as mybir
import concourse.tile as tile

f32 = mybir.dt.float32
u16 = mybir.dt.uint16
i16 = mybir.dt.int16

NPG = 2048          # points per graph
KNN = 20            # neighbors
G = 2               # graphs per core
NCORES = 8
NT = NPG // 128     # 16 i-tiles per graph
NB = NPG // 512     # 4 moving-dim blocks per 2048
NEG = -1.0e30
SLOPE = 0.02
LATENT = 1024
AluOp = mybir.AluOpType
Act = mybir.ActivationFunctionType

LAYERS = [(3, 64), (64, 64), (64, 128), (128, 256)]


def _mm(nc, out, lhsT, rhs, start, stop):
    nc.tensor.matmul(out, lhsT, rhs, start=start, stop=stop)


def emit_selection(nc, pools, ts, i_all, mode="seg"):
    """Top-24 indices (descending) of each row of ts [128, 2048] -> i_all
    [128, 24] u16. Uses ranks 1..20 downstream."""
    selp = pools["sel"]
    if mode == "flat":
        ss = selp.tile([128, NPG], f32, tag="sel_ss")
        v = selp.tile([128, 24], f32, tag="sel_v")
        nc.vector.max(out=v[:, 0:8], in_=ts)
        nc.vector.max_index(out=i_all[:, 0:8], in_max=v[:, 0:8], in_values=ts)
        nc.vector.match_replace(out=ss, in_to_replace=v[:, 0:8], in_values=ts,
                                imm_value=NEG)
        nc.vector.max(out=v[:, 8:16], in_=ss)
        nc.vector.max_index(out=i_all[:, 8:16], in_max=v[:, 8:16], in_values=ss)
        nc.vector.match_replace(out=ss, in_to_replace=v[:, 8:16], in_values=ss,
                                imm_value=NEG)
        nc.vector.max(out=v[:, 16:24], in_=ss)
        nc.vector.max_index(out=i_all[:, 16:24], in_max=v[:, 16:24], in_values=ss)
    else:
        cand = selp.tile([128, 128], f32, tag="sel_cand")
        v = selp.tile([128, 24], f32, tag="sel_v")
        for s in range(16):
            nc.vector.max(out=cand[:, 8 * s:8 * s + 8],
                          in_=ts[:, 128 * s:128 * (s + 1)])
        nc.vector.max(out=v[:, 0:8], in_=cand)
        nc.vector.match_replace(out=cand, in_to_replace=v[:, 0:8],
                                in_values=cand, imm_value=NEG)
        nc.vector.max(out=v[:, 8:16], in_=cand)
        nc.vector.match_replace(out=cand, in_to_replace=v[:, 8:16],
                                in_values=cand, imm_value=NEG)
        nc.vector.max(out=v[:, 16:24], in_=cand)
        nc.vector.max_index(out=i_all[:, 0:8], in_max=v[:, 0:8], in_values=ts)
        nc.vector.max_index(out=i_all[:, 8:16], in_max=v[:, 8:16], in_values=ts)
        nc.vector.max_index(out=i_all[:, 16:24], in_max=v[:, 16:24], in_values=ts)


def emit_layer(nc, tc, pools, consts, lidx, x_sb, sel_mode):
    """One dynamic EdgeConv layer for one graph, channel-major layout.

    x_sb: SBUF [C, 2048] fp32. Returns list of [<=128, 2048] fp32
    channel-chunk outputs (1 chunk if O<=128 else 2).
    """
    C, O = LAYERS[lidx]
    ones = consts["ones"]
    negcol = consts["negcol"]      # [128, 2048] of -1.0
    wdw = consts["wdw"][lidx]      # [C, O]
    wdb = consts["wdb"][lidx]      # [1, O] bias row
    wj = consts["wj"][lidx]        # [C, O]
    work = pools["work"]
    bigps = pools["bigps"]
    dram = pools["dram"]
    nocs = (O + 127) // 128

    # ---- sq = x * x (for the -|x_j|^2 matmul term)
    sq = work.tile([128, NPG], f32, tag="sq")
    nc.vector.tensor_tensor(out=sq[0:C, :], in0=x_sb, in1=x_sb, op=AluOp.mult)

    # ---- rhs2x = 2 * x
    rhs2x = work.tile([128, NPG], f32, tag="rhs2x")
    nc.scalar.activation(out=rhs2x[0:C, :], in_=x_sb, func=Act.Copy, scale=2.0)

    # ---- projections: aT = (x@Wd + b)^T, cT = (x@Wj)^T, both [O, 2048]
    a_parts, c_parts = [], []
    for oc in range(nocs):
        ow = min(128, O - 128 * oc)
        osl = slice(128 * oc, 128 * oc + ow)
        cT_ps = bigps.tile([128, NPG], f32, tag="big_ps")
        for nb in range(NB):
            jsl = slice(512 * nb, 512 * (nb + 1))
            _mm(nc, cT_ps[0:ow, jsl], wj[:, osl], x_sb[:, jsl],
                start=True, stop=True)
        cT = work.tile([128, NPG], f32, tag=f"cT{oc}", name=f"cT{oc}")
        nc.scalar.activation(out=cT[0:ow, :], in_=cT_ps[0:ow, :], func=Act.Copy)
        c_parts.append(cT)

        aT_ps = bigps.tile([128, NPG], f32, tag="big_ps")
        for nb in range(NB):
            jsl = slice(512 * nb, 512 * (nb + 1))
            _mm(nc, aT_ps[0:ow, jsl], wdw[:, osl], x_sb[:, jsl],
                start=True, stop=False)
            _mm(nc, aT_ps[0:ow, jsl], wdb[:, osl], ones[:, jsl],
                start=False, stop=True)
        aT = work.tile([128, NPG], f32, tag=f"aT{oc}", name=f"aT{oc}")
        nc.scalar.activation(out=aT[0:ow, :], in_=aT_ps[0:ow, :], func=Act.Copy)
        a_parts.append(aT)

    # ---- t-matmul + top-k selection per i-tile ---------------------------
    wr_d = dram.tile([NPG, KNN], u16, tag="wr_d")
    for T in range(NT):
        t_ps = bigps.tile([128, NPG], f32, tag="big_ps")
        xsl = slice(128 * T, 128 * (T + 1))
        for nb in range(NB):
            jsl = slice(512 * nb, 512 * (nb + 1))
            _mm(nc, t_ps[:, jsl], x_sb[:, xsl], rhs2x[0:C, jsl],
                start=True, stop=False)
            _mm(nc, t_ps[:, jsl], negcol[0:C, xsl], sq[0:C, jsl],
                start=False, stop=True)
        ts = work.tile([128, NPG], f32, tag="ts")
        nc.scalar.activation(out=ts, in_=t_ps, func=Act.Copy)
        i_all = pools["sel"].tile([128, 24], u16, tag="i_all")
        emit_selection(nc, pools, ts, i_all, mode=sel_mode)
        nc.sync.dma_start(out=wr_d[128 * T:128 * (T + 1), :],
                          in_=i_all[:, 0:KNN])

    # ---- index readback in wrapped-16 + replicated form ------------------
    idx_all = work.tile([128, KNN, 128], i16, tag="idx_all")
    wr_wrapped = wr_d[:, :].rearrange("(s p) k -> p k s", p=16).bitcast(i16)
    for grp in range(8):
        nc.sync.dma_start(out=idx_all[16 * grp:16 * (grp + 1), :, :],
                          in_=wr_wrapped)

    # ---- gather + max aggregation (ap_gather per k) ----------------------
    outs = []
    for oc in range(nocs):
        ow = min(128, O - 128 * oc)
        chk = (ow + 15) // 16 * 16
        m = work.tile([128, NPG], f32, tag=f"m{oc}", name=f"m{oc}")
        for k in range(KNN):
            g = pools["gather"].tile([128, NPG], f32, tag="g_sb")
            nc.gpsimd.ap_gather(
                out_ap=g[0:chk, :], in_ap=c_parts[oc][0:chk, :],
                idxs_ap=idx_all[0:chk, k, :], channels=chk,
                num_elems=NPG, d=1, num_idxs=NPG)
            if k == 0:
                nc.any.tensor_copy(m[0:ow, :], g[0:ow, :])
            else:
                nc.any.tensor_tensor(out=m[0:ow, :], in0=m[0:ow, :],
                                     in1=g[0:ow, :], op=AluOp.max)
        xtag = ("xnA", "xnB", "xnA", "xnB", "xnC")[
            lidx if lidx < 3 else 3 + oc]
        xn = work.tile([128, NPG], f32, tag=xtag, name=f"xn{lidx}_{oc}")
        nc.vector.tensor_tensor(out=xn[0:ow, :], in0=a_parts[oc][0:ow, :],
                                in1=m[0:ow, :], op=AluOp.add)
        nc.vector.scalar_tensor_tensor(out=xn[0:ow, :], in0=xn[0:ow, :],
                                       scalar=SLOPE, in1=xn[0:ow, :],
                                       op0=AluOp.mult, op1=AluOp.max)
        outs.append(xn)
    return outs


def build_nc(sel_mode="seg"):
    nc = bacc.Bacc()
    posT = nc.declare_dram_parameter("posT", [3, G * NPG], f32, isOutput=False)
    wd_d, wj_d = [], []
    for l, (C, O) in enumerate(LAYERS):
        wd_d.append(nc.declare_dram_parameter(f"wd{l}", [C + 1, O], f32,
                                              isOutput=False))
        wj_d.append(nc.declare_dram_parameter(f"wj{l}", [C, O], f32,
                                              isOutput=False))
    wl_d = nc.declare_dram_parameter("wl", [512, LATENT], f32, isOutput=False)
    bl_d = nc.declare_dram_parameter("bl", [1, LATENT], f32, isOutput=False)
    out_d = nc.declare_dram_parameter("out", [G, LATENT], f32, isOutput=True)

    with tile.TileContext(nc) as tc, ExitStack() as ctx:
        const = ctx.enter_context(tc.tile_pool(name="const", bufs=1))
        work = ctx.enter_context(tc.tile_pool(name="work", bufs=1))
        selp = ctx.enter_context(tc.tile_pool(name="selp", bufs=2))
        gat = ctx.enter_context(tc.tile_pool(name="gat", bufs=2))
        bigps = ctx.enter_context(tc.tile_pool(name="bigps", bufs=1,
                                               space="PSUM"))
        smallps = ctx.enter_context(tc.tile_pool(name="smallps", bufs=2,
                                                 space="PSUM"))
        dram = ctx.enter_context(tc.tile_pool(name="dram", bufs=1,
                                              space="DRAM"))
        pools = {"work": work, "sel": selp, "gather": gat, "bigps": bigps,
                 "smallps": smallps, "dram": dram}

        ones = const.tile([1, NPG], f32)
        nc.vector.memset(ones, 1.0)
        negcol = const.tile([128, NPG], f32)
        nc.vector.memset(negcol, -1.0)
        wdw, wdb, wj = [], [], []
        for l, (C, O) in enumerate(LAYERS):
            wdw.append(const.tile_from(wd_d[l][0:C, :], name=f"wdw{l}s"))
            wdb.append(const.tile_from(wd_d[l][C:C + 1, :], name=f"wdb{l}s"))
            wj.append(const.tile_from(wj_d[l][:, :], name=f"wj{l}s"))
        wls = const.tile([128, 4, LATENT], f32)
        nc.sync.dma_start(out=wls,
                          in_=wl_d[:, :].rearrange("(c p) n -> p c n", p=128))
        bls = const.tile_from(bl_d[:, :])
        consts = {"ones": ones, "negcol": negcol,
                  "wdw": wdw, "wdb": wdb, "wj": wj}

        g_all = const.tile([128, 4, G], f32)

        for g in range(G):
            x0 = work.tile([128, NPG], f32, tag="x0")
            nc.sync.dma_start(out=x0[0:3, :],
                              in_=posT[:, g * NPG:(g + 1) * NPG])
            x = [x0[0:3, :]]
            for l, (C, O) in enumerate(LAYERS):
                outs = emit_layer(nc, tc, pools, consts, l, x[0], sel_mode)
                x = [o[0:min(128, O - 128 * oc), :]
                     for oc, o in enumerate(outs)]
                if l == 0:
                    nc.vector.tensor_reduce(out=g_all[0:64, 0:1, g],
                                            in_=x[0], axis=mybir.AxisListType.X,
                                            op=AluOp.max)
                elif l == 1:
                    ptmp = selp.tile([64, 1], f32, tag="ptmp")
                    nc.vector.tensor_reduce(out=ptmp, in_=x[0],
                                            axis=mybir.AxisListType.X,
                                            op=AluOp.max)
                    nc.sync.dma_start(out=g_all[64:128, 0:1, g], in_=ptmp)
                elif l == 2:
                    nc.vector.tensor_reduce(out=g_all[:, 1:2, g], in_=x[0],
                                            axis=mybir.AxisListType.X,
                                            op=AluOp.max)
                else:
                    nc.vector.tensor_reduce(out=g_all[:, 2:3, g], in_=x[0],
                                            axis=mybir.AxisListType.X,
                                            op=AluOp.max)
                    nc.vector.tensor_reduce(out=g_all[:, 3:4, g], in_=x[1],
                                            axis=mybir.AxisListType.X,
                                            op=AluOp.max)

        out_sb = const.tile([G, LATENT], f32)
        for nb in range(LATENT // 512):
            po = smallps.tile([G, 512], f32, tag="po")
            nsl = slice(512 * nb, 512 * (nb + 1))
            for kc in range(4):
                _mm(nc, po, g_all[:, kc, :], wls[:, kc, nsl],
                    start=(kc == 0), stop=False)
            _mm(nc, po, ones[:, 0:G], bls[:, nsl], start=False, stop=True)
            nc.scalar.activation(out=out_sb[:, nsl], in_=po, func=Act.Relu)
        nc.sync.dma_start(out=out_d[:, :], in_=out_sb)

    nc.finalize()
    return nc


# ---------------------------------------------------------------------------
_NC_CACHE = {}


def _get_nc(sel_mode="seg"):
    if sel_mode not in _NC_CACHE:
        _NC_CACHE[sel_mode] = build_nc(sel_mode)
    return _NC_CACHE[sel_mode]


def make_in_maps(inputs):
    pos = np.ascontiguousarray(np.asarray(inputs["pos"], dtype=np.float32))
    Ws = [np.asarray(inputs[f"W{i}"], np.float32) for i in range(1, 5)]
    bs = [np.asarray(inputs[f"b{i}"], np.float32) for i in range(1, 5)]
    wl = np.ascontiguousarray(np.asarray(inputs["Wl"], np.float32))
    bl = np.ascontiguousarray(np.asarray(inputs["bl"], np.float32)[None, :])
    base = {"wl": wl, "bl": bl}
    for l, (C, O) in enumerate(LAYERS):
        W, b = Ws[l], bs[l]
        base[f"wd{l}"] = np.ascontiguousarray(
            np.concatenate([W[:C] - W[C:], b[None, :]], axis=0))
        base[f"wj{l}"] = np.ascontiguousarray(W[C:])
    in_maps = []
    for c in range(NCORES):
        m = dict(base)
        m["posT"] = np.ascontiguousarray(
            pos[c * G * NPG:(c + 1) * G * NPG].T)
        in_maps.append(m)
    return in_maps


def kernel(**inputs) -> np.ndarray:
    from concourse.bass_utils import run_bass_kernel_spmd
    nc = _get_nc()
    in_maps = make_in_maps(inputs)
    res = run_bass_kernel_spmd(nc, in_maps, list(range(NCORES)))
    return np.concatenate([r["out"] for r in res.results], axis=0)


if __name__ == "__main__":
    nc = build_nc("seg")
    print("build OK")

